# revision 62
# baseline (speedup 1.0000x reference)
"""DDiT attention block on 8 trn2 NeuronCores — v2 (no collectives).

Sharding: data-parallel over batch (cores 0-3 -> batch 0, cores 4-7 ->
batch 1) x tensor-parallel over heads (4 heads/core, Megatron-style:
W_qkv rows and W_out columns sharded). Each core computes a PARTIAL
output [T, C] = (its 4 heads' attention) @ W_out[:, its dims].T; the
host sums the 4 partials per batch. No device collectives at all.

Per core (1 batch, 4 heads as 2 pairs, T=2048, C=1024, D=64):
  q,k   = per-pair [128, T] tiles (2 heads stacked on partitions)
  v_aug = per s-tile [128, 4*(64 v | 64 ones)]  (ones give the softmax
          denominator replicated across 64 partitions in the attn@v out)
  per (pair, 512-wide t-slice, 128-row s-tile):
    st   = k_h.T @ q_h for both heads -> [128, 2x512] PSUM
    ste  = exp(st/8) fp16 (one ACT instr, 1024 wide)
    yt  += v_aug_h.T @ ste_h -> [128, 512]: rows 0-63 y, 64-127 denom
  ytn   = yt[0:64]/yt[64:128] (DVE divide) -> fp16, pair-stacked [128, T]
  out  += ytn_pair.T @ wo_pair (K=128, accumulated across pairs via DVE)

All matmuls fp16 (fp8 fails the 2e-2 budget: measured 2.2e-2 with fp8
ste alone). The schedule keeps the PE queue dense: projections and
out-proj tiles are emitted as filler between attention steps so the PE
never waits on the exp stream, and exp (ACT, ~137us) hides under the
PE's ~165us of matmul rows.
"""

import os
import sys
from collections import deque

sys.path.insert(0, "/opt/trn_rl_repo")

import numpy as np

import concourse.bass as bass
import concourse.mybir as mybir
import concourse.tile as tile_mod
from concourse.tile import TileContext
from concourse.vector_clock import ScopedClock

F32 = mybir.dt.float32
F16 = mybir.dt.float16
AF = mybir.ActivationFunctionType

B, T, C = 2, 2048, 1024
H, D = 16, 64
NCORES = 8
GROUP = 4            # cores per batch group (tensor-parallel degree)
HPC = H // GROUP     # heads per core = 4
KT = C // 128        # 8 contraction tiles
NT = T // 512        # 4 t-slices of 512
ST = T // 128        # 16 s-tiles of 128
TT = T // 128        # 16 out t-tiles of 128

# ---------------------------------------------------------------------------
# walrus workarounds: this build rejects >1 sync-wait command per
# instruction. Move excess waits onto standalone event-semaphore nops on the
# same engine queue (equivalent to raw-bass wait_ge + op).
# ---------------------------------------------------------------------------
_WAITSPLIT_CTR = [0]


def _split_excess_waits(nc: bass.Bass, limit: int = 1) -> int:
    moved = 0
    for f in nc.m.functions:
        for bb in f.blocks:
            insts = bb.instructions
            i = 0
            while i < len(insts):
                inst = insts[i]
                si = inst.sync_info
                if si is not None and si.on_wait and len(si.on_wait) > limit:
                    waits = list(si.on_wait)
                    si.on_wait = waits[:limit]
                    for w in waits[limit:]:
                        _WAITSPLIT_CTR[0] += 1
                        moved += 1
                        ev = mybir.InstEventSemaphore(
                            name=f"I-waitsplit-{_WAITSPLIT_CTR[0]}",
                            engine=inst.engine,
                            ins=[],
                            outs=[],
                            sync_info=mybir.SyncInfo(on_wait=[w], on_update=[]),
                        )
                        insts.insert(i, ev)
                        i += 1
                i += 1
    return moved


def _patched_drain_and_barrier(self, tick_clock, wait_clock):
    nc = self.nc
    nop0 = nc.sync.nop(nofuse=True, hint="tile_exit_waits")
    wait_clock.add_sem_waits(nop0.ins, ScopedClock({None: tick_clock.global_clock}))
    nc.sync.drain()
    nc.all_engine_barrier()
    assert self.sems is not None
    popped = nc._tile_sem_poison_stack.pop()
    assert popped is self._sem_poison
    nc.clear_and_free_semaphores(list(self.sems.allocated().values()))
    nc.all_engine_barrier()


def _install_ntff_shim():
    """Provide antenv.axon_hooks (absent in this image) so trace=True can
    reach the libaxon NTFF profiler."""
    import types

    if "antenv.axon_hooks" in sys.modules:
        return
    hook = None
    try:
        sys.path.insert(0, "/root/.axon_site")
        from trn_agent_boot.trn_boot import _ntff_profile_via_ctypes

        so_path = "/opt/axon/libaxon_pjrt.so"
        if os.path.exists(so_path):
            hook = _ntff_profile_via_ctypes(so_path)
    except Exception:
        hook = None
    mod = types.ModuleType("antenv.axon_hooks")
    mod.get_axon_ntff_profile_hook = lambda: hook
    mod.set_axon_ntff_profile_hook = lambda h: None
    sys.modules["antenv.axon_hooks"] = mod


tile_mod.TileContext._drain_and_barrier = _patched_drain_and_barrier
_install_ntff_shim()


# ---------------------------------------------------------------------------
# device program (identical on all 8 cores; per-core data differs)
# ---------------------------------------------------------------------------
def _build() -> bass.Bass:
    nc = bass.Bass(trn_type="TRN2", target_bir_lowering=False, num_devices=NCORES)

    # host pre-arranges inputs partition-major so every DMA descriptor
    # line is >=8KB contiguous (1KB lines made the load descriptor-bound)
    xc = nc.dram_tensor("xc", [NT * 128, KT * 512], F16, kind="ExternalInput")
    wqk = nc.dram_tensor("wqk", [128, KT * 512], F16, kind="ExternalInput")
    wv = nc.dram_tensor("wv", [128, KT * 256], F16, kind="ExternalInput")
    wo = nc.dram_tensor("wo", [128, 2 * C], F16, kind="ExternalInput")
    outp_d = [
        nc.dram_tensor(f"outp{j}", [T, C], F16, kind="ExternalOutput")
        for j in range(2)
    ]
    out_v = [
        outp_d[j].rearrange("(tt p) f -> tt p f", p=128) for j in range(2)
    ]

    with TileContext(nc) as tc:
        with (
            tc.tile_pool(name="pw", bufs=1) as pw,
            tc.tile_pool(name="px", bufs=1) as px,
            tc.tile_pool(name="pqk", bufs=1) as pqk,
            tc.tile_pool(name="pv", bufs=1) as pv,
            tc.tile_pool(name="pst", bufs=3) as pst,
            tc.tile_pool(name="pyn", bufs=1) as pyn,
            tc.tile_pool(name="ps_st", bufs=2, space="PSUM") as ps_st,
            tc.tile_pool(name="ps_yt", bufs=1, space="PSUM") as ps_yt,
            tc.tile_pool(name="ps_mm", bufs=2, space="PSUM") as ps_mm,
        ):
            # ---- persistent SBUF tiles -------------------------------------
            # x layout: [128, (n-chunk, k-tile, 512)] so each 512-wide
            # n-chunk DMA is one contiguous 8KB run per partition
            # wqk layout: [128, (m-tile, k-tile, 128)] so each m-block is one
            # contiguous DMA, loadable in first-use order
            wqk_sb2 = pw.tile([128, KT * 512], F16, name="wqk_sb")
            wv_sb2 = pw.tile([128, KT * 256], F16, name="wv_sb")
            wo_sb2 = pw.tile([128, 2 * C], F16, name="wo_sb")
            x_sb2 = px.tile([128, NT * KT * 512], F16, name="x_sb")
            wv_sb = [wv_sb2[:, 256 * k : 256 * (k + 1)] for k in range(KT)]
            wo_sb = [wo_sb2[:, C * j : C * (j + 1)] for j in range(2)]

            def wqk_slice(m, k):
                base = 1024 * m + 128 * k
                return wqk_sb2[:, base : base + 128]

            def x_chunk(k, n, a, b):
                """x_sb view of k-tile k, t-range [512n+a, 512n+b)."""
                base = 512 * (KT * n + k)
                return x_sb2[:, base + a : base + b]
            # qk m-tiles: 0=q01, 1=q23, 2=k01, 3=k23 (2 heads stacked)
            qk_sb = [pqk.tile([128, T], F16, name=f"qk{m}") for m in range(4)]
            # v per s-tile: 4 x [64 v-cols | 64 ones]
            v_sb = [pv.tile([128, 512], F16, name=f"v{s}") for s in range(ST)]
            ytn = [pyn.tile([128, T], F16, name=f"ytn{j}") for j in range(2)]

            # ---- input DMAs, ordered by first use, 8KB+ descriptor lines ---
            def wqk_dma(m):
                nc.sync.dma_start(
                    out=wqk_sb2[:, 1024 * m : 1024 * (m + 1)],
                    in_=wqk[:, 1024 * m : 1024 * (m + 1)],
                )

            def x_dma(n, k0, k1):
                lo, hi = KT * 512 * n + 512 * k0, KT * 512 * n + 512 * k1
                nc.sync.dma_start(
                    out=x_sb2[:, lo:hi],
                    in_=xc[128 * n : 128 * (n + 1), 512 * k0 : 512 * k1],
                )

            x_dma(0, 0, 4)
            wqk_dma(2)          # k01
            x_dma(0, 4, 8)
            wqk_dma(0)          # q01
            x_dma(1, 0, 8)
            nc.sync.dma_start(out=wv_sb2[:], in_=wv[:])
            wqk_dma(1)          # q23
            wqk_dma(3)          # k23
            nc.sync.dma_start(out=wo_sb2[:], in_=wo[:])
            x_dma(2, 0, 8)
            x_dma(3, 0, 8)

            # ones columns of v_aug (DVE; Pool carries the proj copies)
            for s in range(ST):
                vv = v_sb[s].rearrange("p (h x) -> p h x", x=128)
                nc.gpsimd.memset(vv[:, :, 64:128], 1.0)

            # ---- emission helpers -----------------------------------------
            def proj_chunk(m, n):
                """qk projection m-tile (128 features) for one 512 t-chunk."""
                nsl = slice(512 * n, 512 * (n + 1))
                ps = ps_mm.tile([128, 512], F32, name="mm_ps", tag="mm")
                for k in range(KT):
                    nc.tensor.matmul(
                        ps[:],
                        wqk_slice(m, k),
                        x_chunk(k, n, 0, 512),
                        start=(k == 0),
                        stop=(k == KT - 1),
                    )
                nc.vector.tensor_copy(out=qk_sb[m][:, nsl], in_=ps[:])

            def vproj_tile(s):
                """v projection for one 128-row s-tile -> v_aug slots."""
                ps = ps_mm.tile([128, 512], F32, name="mm_ps", tag="mm")
                a = 128 * (s % 4)
                for k in range(KT):
                    nc.tensor.matmul(
                        ps[:, 0:256],
                        x_chunk(k, s // 4, a, a + 128),
                        wv_sb[k][:],
                        start=(k == 0),
                        stop=(k == KT - 1),
                    )
                vv = v_sb[s].rearrange("p (h x) -> p h x", x=128)
                nc.vector.tensor_copy(
                    out=vv[:, :, 0:64],
                    in_=ps[:, 0:256].rearrange("p (h d) -> p h d", d=64),
                )

            def outproj_tile(j, t, act_copy=False):
                """out-proj for one 128-t tile of pair j's partial: two
                512-wide matmuls, fp16 stage, one DMA. At pair boundaries
                the psum->sbuf copies run on ACT to keep DVE clear."""
                osb = pst.tile([128, C], F16, name="osb", tag="osb", bufs=3)
                for o in range(2):
                    osl = slice(512 * o, 512 * (o + 1))
                    ps = ps_mm.tile([128, 512], F32, name="mm_ps", tag="mm")
                    nc.tensor.matmul(
                        ps[:],
                        ytn[j][:, 128 * t : 128 * (t + 1)],
                        wo_sb[j][:, osl],
                        start=True,
                        stop=True,
                    )
                    if act_copy:
                        nc.scalar.activation(
                            out=osb[:, osl], in_=ps[:], func=AF.Copy
                        )
                    else:
                        nc.vector.tensor_copy(out=osb[:, osl], in_=ps[:])
                nc.sync.dma_start(out=out_v[j][t], in_=osb[:])

            def pop_filler(act_copy=False):
                if filler:
                    jt = filler.popleft()
                    outproj_tile(jt[0], jt[1], act_copy)

            filler = deque()

            # ---- startup: enough proj for (j0, n0) to begin ----------------
            proj_chunk(2, 0)   # k01 s-tiles 0-3
            proj_chunk(0, 0)   # q01 t 0-511
            vproj_tile(0)
            vproj_tile(1)
            # remaining projections: (m, n) -> run at (j, n, s); chosen so
            # every chunk lands well before its first reader
            proj_sched = {
                (0, 0, 1): (2, 1), (0, 0, 5): (2, 2), (0, 0, 9): (2, 3),
                (0, 0, 12): (0, 1),
                (0, 1, 4): (1, 0), (0, 1, 6): (1, 1),
                (0, 1, 8): (1, 2), (0, 1, 10): (1, 3),
                (0, 1, 12): (0, 2),
                (0, 2, 4): (3, 0), (0, 2, 6): (3, 1),
                (0, 2, 8): (3, 2), (0, 2, 10): (3, 3),
                (0, 2, 12): (0, 3),
            }

            # ---- attention + interleaved filler ---------------------------
            for j in range(2):
                for n in range(NT):
                    nsl = slice(512 * n, 512 * (n + 1))
                    yt_ps = {
                        hi: ps_yt.tile(
                            [128, 512], F32, name=f"yt{hi}", tag=f"yt{hi}"
                        )
                        for hi in range(2)
                    }
                    for s in range(ST):
                        first = j == 0 and n == 0
                        st = ps_st.tile([128, 1024], F32, name="st", tag="st")
                        for hi in range(2):
                            psl = slice(64 * hi, 64 * (hi + 1))
                            nc.tensor.matmul(
                                st[:, 512 * hi : 512 * (hi + 1)],
                                qk_sb[2 + j][psl, 128 * s : 128 * (s + 1)],
                                qk_sb[j][psl, nsl],
                                start=True,
                                stop=True,
                            )
                        ste = pst.tile(
                            [128, 1024], F16, name="ste", tag="ste", bufs=4
                        )
                        nc.scalar.activation(
                            out=ste[:], in_=st[:], func=AF.Exp, scale=0.125
                        )
                        busy_step = False
                        if (j, n, s) in proj_sched:
                            proj_chunk(*proj_sched[(j, n, s)])
                            busy_step = True
                        if first and s <= 13:
                            vproj_tile(s + 2)
                            busy_step = True
                        if not busy_step:
                            if s >= 14:
                                # boundary: copies go via ACT, DVE stays
                                # clear for the yt drain
                                pop_filler(act_copy=True)
                            else:
                                # hold back a few tiles so the late-j1
                                # boundaries never run dry
                                reserve = 3 if j == 1 else 0
                                for _ in range(2 if s in (0, 1) else 1):
                                    if len(filler) > reserve:
                                        pop_filler()
                        for hi in range(2):
                            h = 2 * j + hi
                            nc.tensor.matmul(
                                yt_ps[hi][:],
                                v_sb[s][:, 128 * h : 128 * (h + 1)],
                                ste[:, 512 * hi : 512 * (hi + 1)],
                                start=(s == 0),
                                stop=(s == ST - 1),
                            )
                    # pack both heads: y rows -> yt_y, denom rows -> yt_l,
                    # one reciprocal + one multiply, all base-partition 0
                    yt_y = pst.tile([128, 512], F32, name="yt_y", tag="yty", bufs=2)
                    yt_l = pst.tile([128, 512], F32, name="yt_l", tag="ytl", bufs=2)
                    for hi in range(2):
                        psl = slice(64 * hi, 64 * (hi + 1))
                        nc.vector.tensor_copy(
                            out=yt_y[psl, :], in_=yt_ps[hi][0:64, :]
                        )
                        nc.vector.tensor_copy(
                            out=yt_l[psl, :], in_=yt_ps[hi][64:128, :]
                        )
                    lnl = pst.tile([128, 512], F32, name="lnl", tag="lnl", bufs=2)
                    nc.scalar.activation(out=lnl[:], in_=yt_l[:], func=AF.Ln)
                    r2 = pst.tile([128, 512], F32, name="r2", tag="r2", bufs=2)
                    nc.scalar.activation(
                        out=r2[:], in_=lnl[:], func=AF.Exp, scale=-1.0
                    )
                    nc.vector.tensor_tensor(
                        out=ytn[j][:, nsl],
                        in0=yt_y[:],
                        in1=r2[:],
                        op=mybir.AluOpType.mult,
                    )
                    # out-proj tiles for this n-slice become filler
                    for t in range(4 * n, 4 * (n + 1)):
                        filler.append((j, t))
            k = 0
            while filler:
                pop_filler(act_copy=(k % 2 == 0))
                k += 1

    _split_excess_waits(nc)
    return nc


_NC_CACHE = []
LAST_RESULTS = None


def kernel(**inputs: np.ndarray) -> np.ndarray:
    global LAST_RESULTS
    from concourse.bass_utils import run_bass_kernel_spmd

    x = np.asarray(inputs["x"], dtype=np.float32)
    W_qkv = np.asarray(inputs["W_qkv"], dtype=np.float32)
    W_out = np.asarray(inputs["W_out"], dtype=np.float32)

    in_maps = []
    for c in range(NCORES):
        g, r = divmod(c, GROUP)
        h0 = HPC * r
        q_rows = W_qkv[64 * h0 : 64 * (h0 + HPC)]
        k_rows = W_qkv[C + 64 * h0 : C + 64 * (h0 + HPC)]
        v_rows = W_qkv[2 * C + 64 * h0 : 2 * C + 64 * (h0 + HPC)]
        xT_c = x[g].T  # [C, T]
        xc = xT_c.reshape(KT, 128, NT, 512).transpose(2, 1, 0, 3)
        wqk_c = np.concatenate([q_rows, k_rows], axis=0).T  # [C, 512]
        wv_c = v_rows.T  # [C, 256]
        dims = slice(64 * h0, 64 * (h0 + HPC))
        wo_c = W_out[:, dims].T  # [256, C]
        im = {
            "xc": np.ascontiguousarray(
                xc.reshape(NT * 128, KT * 512)
            ).astype(np.float16),
            "wqk": np.ascontiguousarray(
                wqk_c.reshape(KT, 128, 4, 128)
                .transpose(1, 2, 0, 3)
                .reshape(128, -1)
            ).astype(np.float16),
            "wv": np.ascontiguousarray(
                wv_c.reshape(KT, 128, 256).transpose(1, 0, 2).reshape(128, -1)
            ).astype(np.float16),
            "wo": np.ascontiguousarray(
                wo_c.reshape(2, 128, C).transpose(1, 0, 2).reshape(128, -1)
            ).astype(np.float16),
        }
        in_maps.append(im)

    if not _NC_CACHE:
        _NC_CACHE.append(_build())
    nc = _NC_CACHE[0]

    trace = os.environ.get("KERNEL_TRACE", "0") == "1"
    trace_cores = None
    if trace:
        tc_env = os.environ.get("KERNEL_TRACE_CORES", "0")
        trace_cores = [int(t) for t in tc_env.split(",")]
    res = run_bass_kernel_spmd(
        nc,
        in_maps,
        core_ids=list(range(NCORES)),
        trace=trace,
        trace_cores=trace_cores,
    )
    LAST_RESULTS = res

    out = np.zeros((B, T, C), dtype=np.float32)
    for c in range(NCORES):
        g, _ = divmod(c, GROUP)
        out[g] += res.results[c]["outp0"].astype(np.float32)
        out[g] += res.results[c]["outp1"].astype(np.float32)
    return out
